# revision 3
# baseline (speedup 1.0000x reference)
"""Batched attention kernel for Trainium2, SPMD over 8 NeuronCores.

Computes, for inputs K, V, Q of shape [16, 2048, 256] (f32):
    A = softmax(Q @ K^T / sqrt(256), axis=-1)      # [16, 2048, 2048]
    R = concat(A @ V, Q, axis=-1)                  # [16, 2048, 512]
and returns (R, A), matching the reference.

Sharding: batch dim across the 8 cores (2 batches per core), fully local.

Per-core dataflow (per batch):
  prep: load K/Q tiles f32 -> PE transpose -> evict-cast to bf16 K^T/Q^T in
        SBUF; V cast to bf16; Q f32 tiles DMA'd straight out to R[..., D:].
  main (16 q-tiles of 128 rows):
        S = Q@K^T into PSUM (bf16 matmuls, f32 accum)
        E = exp(S/16) via ScalarE with free row-sum accumulation
        A_tile = E * (1/rowsum) via ScalarE, DMA out
        E^T via PE transposes + DVE evict (bf16)
        O = E^T.T @ V accumulated in PSUM, scaled by 1/rowsum, DMA to R[..., :D]
  Softmax max-subtraction is skipped: scores ~ N(0,1), no overflow risk.
"""

import numpy as np

B, T, D = 16, 2048, 256
NCORES = 8
BPC = B // NCORES   # batches per core
NT = T // 128       # 16 row-tiles per sequence
ND = D // 128       # 2 contraction chunks

SCALE = 1.0 / np.sqrt(np.float32(D)).astype(np.float32)  # 1/16


def build_nc(
    n_schunks=2,          # S psum chunks per q-tile (each T//n_schunks wide)
    spsum_bufs=2,
    tpsum_bufs=2,
    opsum_bufs=2,
    e_bufs=3,
    et_bufs=2,
    a_bufs=3,
    anorm_engine="scalar",  # "scalar" or "vector"
    et_mode="pe",           # "pe" or "dma" (xbar transpose)
):
    from contextlib import ExitStack
    import concourse.bacc as bacc
    import concourse.tile as tile
    from concourse import mybir, masks

    f32 = mybir.dt.float32
    bf16 = mybir.dt.bfloat16
    AF = mybir.ActivationFunctionType

    CW = T // n_schunks            # chunk width in k
    assert CW % 512 == 0

    nc = bacc.Bacc(None, target_bir_lowering=False)
    Kd = nc.declare_dram_parameter("K", [BPC, T, D], f32, isOutput=False)
    Vd = nc.declare_dram_parameter("V", [BPC, T, D], f32, isOutput=False)
    Qd = nc.declare_dram_parameter("Q", [BPC, T, D], f32, isOutput=False)
    Rd = nc.declare_dram_parameter("R", [BPC, T, 2 * D], f32, isOutput=True)
    Ad = nc.declare_dram_parameter("A", [BPC, T, T], f32, isOutput=True)

    with tile.TileContext(nc) as tc, ExitStack() as ctx:
        singles = ctx.enter_context(tc.tile_pool(name="singles", bufs=1))
        batchp = ctx.enter_context(tc.tile_pool(name="batchp", bufs=2))
        loads = ctx.enter_context(tc.tile_pool(name="loads", bufs=4))
        epool = ctx.enter_context(tc.tile_pool(name="epool", bufs=e_bufs))
        etpool = ctx.enter_context(tc.tile_pool(name="etpool", bufs=et_bufs))
        apool = ctx.enter_context(tc.tile_pool(name="apool", bufs=a_bufs))
        rpool = ctx.enter_context(tc.tile_pool(name="rpool", bufs=3))
        small = ctx.enter_context(tc.tile_pool(name="small", bufs=4))
        spsum = ctx.enter_context(tc.tile_pool(name="spsum", bufs=spsum_bufs, space="PSUM"))
        tpsum = ctx.enter_context(tc.tile_pool(name="tpsum", bufs=tpsum_bufs, space="PSUM"))
        opsum = ctx.enter_context(tc.tile_pool(name="opsum", bufs=opsum_bufs, space="PSUM"))

        ident_f32 = singles.tile([128, 128], f32)
        masks.make_identity(nc, ident_f32)
        ident_bf16 = singles.tile([128, 128], bf16)
        masks.make_identity(nc, ident_bf16)

        for b in range(BPC):
            # ---- prep: K^T, Q^T (bf16), V bf16, R right half ----
            KT = batchp.tile([128, ND, NT, 128], bf16, tag="KT")
            QT = batchp.tile([128, ND, NT, 128], bf16, tag="QT")
            Vb = batchp.tile([128, NT, D], bf16, tag="Vb")

            for t in range(NT):
                kt_f32 = loads.tile([128, D], f32, tag="kload")
                nc.sync.dma_start(out=kt_f32, in_=Kd[b, t * 128:(t + 1) * 128, :])
                for dc in range(ND):
                    tp = tpsum.tile([128, 2, 128], f32, tag="tp")
                    nc.tensor.transpose(tp[:, 0, :], kt_f32[:, dc * 128:(dc + 1) * 128], ident_f32)
                    nc.vector.tensor_copy(KT[:, dc, t, :], tp[:, 0, :])

                qt_f32 = loads.tile([128, D], f32, tag="qload")
                nc.sync.dma_start(out=qt_f32, in_=Qd[b, t * 128:(t + 1) * 128, :])
                nc.sync.dma_start(out=Rd[b, t * 128:(t + 1) * 128, D:2 * D], in_=qt_f32)
                for dc in range(ND):
                    tp = tpsum.tile([128, 2, 128], f32, tag="tp")
                    nc.tensor.transpose(tp[:, 0, :], qt_f32[:, dc * 128:(dc + 1) * 128], ident_f32)
                    nc.vector.tensor_copy(QT[:, dc, t, :], tp[:, 0, :])

                vt_f32 = loads.tile([128, D], f32, tag="vload")
                nc.sync.dma_start(out=vt_f32, in_=Vd[b, t * 128:(t + 1) * 128, :])
                nc.vector.tensor_copy(Vb[:, t, :], vt_f32)

            # ---- main loop over q-tiles ----
            for qt in range(NT):
                schunks = [
                    spsum.tile([128, CW], f32, tag="schunk", name=f"schunk{i}")
                    for i in range(n_schunks)
                ]
                # S = Q @ K^T (scores unscaled; 1/16 folded into exp)
                for dc in range(ND):
                    for c in range(n_schunks):
                        for h in range(CW // 512):
                            ks = c * CW + h * 512
                            rhs = KT[:, dc, ks // 128:(ks + 512) // 128, :]
                            nc.tensor.matmul(
                                schunks[c][:, h * 512:(h + 1) * 512],
                                QT[:, dc, qt, :],
                                rhs,
                                start=(dc == 0),
                                stop=(dc == ND - 1),
                            )

                E = epool.tile([128, T], bf16, tag="E")
                sumparts = small.tile([128, n_schunks], f32, tag="sp")
                for c in range(n_schunks):
                    nc.scalar.activation(
                        E[:, c * CW:(c + 1) * CW],
                        schunks[c],
                        AF.Exp,
                        scale=float(SCALE),
                        accum_out=sumparts[:, c:c + 1],
                    )
                rinv = small.tile([128, 1], f32, tag="ri")
                if n_schunks == 1:
                    nc.vector.reciprocal(rinv, sumparts)
                else:
                    rowsum = small.tile([128, 1], f32, tag="rs")
                    nc.vector.tensor_reduce(
                        out=rowsum, in_=sumparts,
                        axis=mybir.AxisListType.X, op=mybir.AluOpType.add,
                    )
                    nc.vector.reciprocal(rinv, rowsum)

                Asb = apool.tile([128, T], f32, tag="A")
                if anorm_engine == "scalar":
                    nc.scalar.activation(Asb, E, AF.Copy, scale=rinv)
                else:
                    nc.vector.tensor_scalar_mul(Asb, E, rinv)
                nc.sync.dma_start(out=Ad[b, qt * 128:(qt + 1) * 128, :], in_=Asb)

                # E^T for the O matmul
                ET = etpool.tile([128, NT, 128], bf16, tag="ET")
                if et_mode == "pe":
                    for j in range(NT // 2):
                        tp = tpsum.tile([128, 2, 128], bf16, tag="tp")
                        for u in range(2):
                            kc = 2 * j + u
                            nc.tensor.transpose(
                                tp[:, u, :], E[:, kc * 128:(kc + 1) * 128], ident_bf16
                            )
                        nc.vector.tensor_copy(ET[:, 2 * j:2 * j + 2, :], tp)
                else:
                    nc.sync.dma_start_transpose(out=ET, in_=E)

                Opsum = opsum.tile([128, D], f32, tag="O")
                for kc in range(NT):
                    nc.tensor.matmul(
                        Opsum, ET[:, kc, :], Vb[:, kc, :],
                        start=(kc == 0), stop=(kc == NT - 1),
                    )
                Rl = rpool.tile([128, D], f32, tag="Rl")
                nc.vector.tensor_scalar_mul(Rl, Opsum, rinv)
                nc.sync.dma_start(out=Rd[b, qt * 128:(qt + 1) * 128, 0:D], in_=Rl)

    nc.compile()
    return nc


_cached = {}


def _get_nc(**kw):
    key = tuple(sorted(kw.items()))
    if key not in _cached:
        _cached[key] = build_nc(**kw)
    return _cached[key]


def kernel(K, V, Q, **build_kw):
    from concourse.bass_utils import run_bass_kernel_spmd

    nc = _get_nc(**build_kw)
    K = np.asarray(K, dtype=np.float32)
    V = np.asarray(V, dtype=np.float32)
    Q = np.asarray(Q, dtype=np.float32)
    in_maps = [
        {
            "K": np.ascontiguousarray(K[c * BPC:(c + 1) * BPC]),
            "V": np.ascontiguousarray(V[c * BPC:(c + 1) * BPC]),
            "Q": np.ascontiguousarray(Q[c * BPC:(c + 1) * BPC]),
        }
        for c in range(NCORES)
    ]
    res = run_bass_kernel_spmd(nc, in_maps, core_ids=list(range(NCORES)))
    R = np.concatenate([res.results[c]["R"] for c in range(NCORES)], axis=0)
    A = np.concatenate([res.results[c]["A"] for c in range(NCORES)], axis=0)
    return (R, A)
